# revision 14
# baseline (speedup 1.0000x reference)
"""Trainium2 Bass kernel for CFContrastiveLoss.

Reference semantics (per sample of N=16 options, D=768 dims):
  - L2-normalize option embeddings
  - sim = pairwise cosine sims within the sample (16x16 gram)
  - max_neg[n] = max over negative-labeled columns of sim[n, :]
  - loss = mean over (positive rows of valid samples) of relu(max_neg + 0.3)

Device strategy (pure data parallel over batch, 8 cores):
  - The loss is a global mean, so samples can be assigned to any
    (core, group) slot.  The host packs 8 samples = 128 rows per group
    (128 groups per core), balancing per-group negative counts, and
    permutes each group's rows to [positives | negatives].
  - Labels are known before the program is built (built per call), so
    the schedule uses a fixed moving window: stationary = all 128 rows
    (full-width weights keep FWL on), moving = the last NSTAR columns
    (NSTAR = max negatives in any group, ~66).  This nearly halves
    TensorE streaming vs a full 128x128 gram: only pos-row x neg-col
    sims are computed.  Pos rows caught inside the window are masked.
  - Embeddings are host-normalized, scaled by 8 (a power of two so the
    final division is exact) and quantized to fp8 e4m3; measured
    end-to-end loss rel-err ~1.4e-4 (errors average out over ~52k
    contributing rows).  fp8 halves HBM traffic vs fp16 - the kernel
    is DMA-dispatch-bound.  Matmuls run in normal mode (DoubleRow
    disables FWL and LDWEIGHTS would dominate at our moving size).
  - DMA: only SP/Activation have hardware DGE queues (gpsimd's software
    queue serializes at ~300ns/packet - measured).  Queue dispatch costs
    ~18ns + ~3ns/KB per per-partition line, so each queue carries a
    64-partition range with full 24 KiB lines (super-groups of 32
    groups).  The output is accumulated in one SBUF tile and written by
    a single end-of-kernel DMA per queue half: a per-super-group
    [128 x 32] f32 write would cost 128 tiny packets each (~2.4us of
    queue time) - measured 18us of dispatch in an earlier revision.
  - Masking is folded into the PSUM accumulation as one extra K=9
    matmul of +-128 sentinel outer products (fp8-exact powers of two):
      row 0:   ones x (-128 * ones)            (mask everything ...)
      row 1+s: u_s  x (+128 * v_s)             (... except same-sample
                                                real-negative columns)
    u_s = rows of sample-slot s, v_s = real-negative moving columns of
    sample-slot s.  Sentinels cancel exactly in fp32 PSUM, so kept sims
    are bit-exact; masked entries sit <= sim-128 <= -64 and the host
    relu(max/64 + 0.3) kills them (invalid samples come out 0 for free).
  - Per group one VectorE row-max from PSUM; relu/weight/mean on host.
"""

import os

import numpy as np
import ml_dtypes

import concourse.bass as bass
import concourse.mybir as mybir
from concourse import bacc, tile
from concourse.bass_utils import run_bass_kernel_spmd

FP8 = mybir.dt.float8e4
F32 = mybir.dt.float32
NP_FP8 = ml_dtypes.float8_e4m3

B, N, D = 8192, 16, 768
N_CORES = 8
ROWS = B * N                      # 131072
ROWS_PER_CORE = ROWS // N_CORES   # 16384
SAMPLES_PER_CORE = ROWS_PER_CORE // N   # 1024
GROUPS = ROWS_PER_CORE // 128     # 128 groups of 128 rows (8 samples)
SPG = 128 // N                    # 8 samples per group
KCH = D // 128                    # 6 contraction chunks
SG = 32                           # groups per super-group (one DMA batch)
N_SG = GROUPS // SG               # 4
MASK_K = 1 + SPG                  # 9 live mask matmul rows
SCALE = np.float32(8.0)           # fp8 pre-scale (power of two)
SENT = np.float32(128.0)          # fp8-exact sentinel, > 1.3 * SCALE^2
MARGIN = np.float32(0.3)

_CACHE: dict = {}

LAST_RESULT = None  # BassKernelResults of the most recent device run


def _build_program(nstar: int) -> bass.Bass:
    nc = bacc.Bacc(None)
    et = nc.declare_dram_parameter("et", [N_SG, 128, SG * D], FP8, isOutput=False)
    mk = nc.declare_dram_parameter(
        "mk", [N_SG, MASK_K, SG * (128 + nstar)], FP8, isOutput=False)
    out = nc.declare_dram_parameter("out", [128, GROUPS], F32, isOutput=True)

    mv0 = 128 - nstar  # first moving column within each group

    with tile.TileContext(nc) as tc:
        with (
            tc.tile_pool(name="emb", bufs=3) as emb_pool,
            tc.tile_pool(name="const", bufs=1) as const_pool,
            tc.tile_pool(name="psum", bufs=8, space="PSUM") as psum_pool,
        ):
            # Ping-pong mask tiles (lhs cols | rhs cols per group, one DMA),
            # zero-padded to K=128: a K=9 mask matmul stalls the PE ~100ns
            # per group (stationary partition-size reconfig); dead
            # contraction rows are free.  Memset once at start.
            GW = 128 + nstar
            mk_tiles = []
            for i in range(2):
                mk_t = const_pool.tile([128, SG * GW], FP8, name=f"mk{i}")
                nc.vector.memset(mk_t[:, :], 0.0)
                mk_tiles.append(mk_t)
            wide = const_pool.tile([128, GROUPS], F32, name="wide")

            queues = [nc.sync, nc.scalar]
            for sg in range(N_SG):
                hi = emb_pool.tile([128, SG * D], FP8, tag="hi")
                mk_t = mk_tiles[sg % 2]
                # Column-split halves: whole-line descriptors are the
                # measured best queue shape; each queue's half holds whole
                # groups, so the leading groups' matmuls depend on one
                # queue's transfer only.  Mask DMA rides an alternating
                # queue AFTER the halves (nothing queues behind the
                # startup memsets; masks land with the embeddings).
                H = SG * D // 2
                if sg == 0:
                    # First super-group in column quarters so the leading
                    # groups' matmuls start as soon as their slice lands;
                    # the sg0 mask load goes first (tiny, and the memset
                    # it depends on finishes during queue startup).
                    nc.sync.dma_start(mk_t[:MASK_K, :], mk[sg])
                    Q = SG * D // 4
                    for i in range(2):
                        for q, eng in enumerate(queues):
                            c0 = (q * 2 + i) * Q
                            eng.dma_start(hi[:, c0:c0 + Q], et[sg][:, c0:c0 + Q])
                else:
                    for q, eng in enumerate(queues):
                        eng.dma_start(hi[:, q * H:(q + 1) * H],
                                      et[sg][:, q * H:(q + 1) * H])
                    queues[sg % 2].dma_start(mk_t[:MASK_K, :], mk[sg])
                for gi in range(SG):
                    ps = psum_pool.tile([128, 512], F32)  # one full PSUM bank
                    G = ps[:, 0:nstar]
                    # Mask sentinels first (start=True clears the bank).
                    nc.tensor.matmul(
                        G,
                        mk_t[:, gi * GW:gi * GW + 128],
                        mk_t[:, gi * GW + 128:(gi + 1) * GW],
                        start=True, stop=False,
                    )
                    for k in range(KCH):
                        c0 = (gi * KCH + k) * 128
                        hk = hi[:, c0:c0 + 128]
                        nc.tensor.matmul(
                            G, hk, hk[:, mv0:128], start=False, stop=(k == KCH - 1))
                    g = sg * SG + gi
                    nc.vector.reduce_max(
                        wide[:, g:g + 1], G, axis=mybir.AxisListType.X)
            # Single output write, split across both hardware queues
            # (tiny per-partition lines make per-super-group writes cost
            # ~128 dispatch slots each).
            nc.sync.dma_start(out[0:64, :], wide[0:64, :])
            nc.scalar.dma_start(out[64:128, :], wide[64:128, :])
    nc.finalize()
    return nc


def _pack_groups(negs: np.ndarray) -> np.ndarray:
    """Assign SAMPLES_PER_CORE samples to GROUPS bins of SPG, balancing
    per-bin negative-row totals (greedy LPT).  Returns [GROUPS, SPG].
    Full bins leave the heap (only re-pushed while below capacity), and
    capacity exactly matches the sample count, so the pop always finds
    a non-full bin."""
    import heapq

    order = np.argsort(-negs, kind="stable")
    heap = [(0, g) for g in range(GROUPS)]
    heapq.heapify(heap)
    bins = [[] for _ in range(GROUPS)]
    for i in order:
        tot, g = heapq.heappop(heap)
        bins[g].append(i)
        if len(bins[g]) < SPG:
            heapq.heappush(heap, (tot + int(negs[i]), g))
    return np.array(bins, dtype=np.int64)


def _prep_core(Xq: np.ndarray, lab: np.ndarray, c: int, nstar: int,
               gidx: np.ndarray):
    """Per-core input map.  Xq: [ROWS, D] fp8 (normalized*SCALE), lab flat."""
    r0 = c * ROWS_PER_CORE
    lab_c = lab[r0:r0 + ROWS_PER_CORE].reshape(SAMPLES_PER_CORE, N)

    rows = (gidx[:, :, None] * N + np.arange(N)).reshape(GROUPS, 128)
    glab = lab_c.reshape(-1)[rows]                         # [GROUPS, 128]
    negflag = glab == 0
    order = np.argsort(negflag, axis=1, kind="stable")     # pos first
    prow = np.take_along_axis(rows, order, axis=1)         # [GROUPS, 128]
    mg = (~negflag).sum(axis=1)                            # pos count per group

    sampslot = np.broadcast_to(np.arange(128) // N, (GROUPS, 128))
    samp_p = np.take_along_axis(sampslot, order, axis=1)   # sample slot per col
    isneg_p = np.take_along_axis(negflag, order, axis=1)

    # mask lhsT rows: row 0 = ones; row 1+s = [col is sample-slot s]
    onehot = (samp_p[:, None, :] == np.arange(SPG)[None, :, None])
    mlhs = np.empty((GROUPS, MASK_K, 128), dtype=np.float32)
    mlhs[:, 0, :] = 1.0
    mlhs[:, 1:, :] = onehot
    # mask rhs rows (last nstar cols): row 0 = -SENT;
    # row 1+s = +SENT * [real-negative col of sample-slot s]
    mv0 = 128 - nstar
    mrhs = np.empty((GROUPS, MASK_K, nstar), dtype=np.float32)
    mrhs[:, 0, :] = -SENT
    mrhs[:, 1:, :] = SENT * (onehot[:, :, mv0:] & isneg_p[:, None, mv0:])

    Xp = Xq[r0 + prow]                                     # [GROUPS, 128, D] fp8
    et = np.ascontiguousarray(
        Xp.reshape(N_SG, SG, 128, KCH, 128).transpose(0, 4, 1, 3, 2)
    ).reshape(N_SG, 128, SG * D)
    mkc = np.concatenate([mlhs, mrhs], axis=2)             # [GROUPS, MASK_K, 128+nstar]
    mk8 = np.ascontiguousarray(
        mkc.astype(NP_FP8).reshape(N_SG, SG, MASK_K, 128 + nstar).transpose(0, 2, 1, 3)
    ).reshape(N_SG, MASK_K, SG * (128 + nstar))
    return {"et": et, "mk": mk8}, mg


def kernel(embeddings: np.ndarray, labels: np.ndarray) -> np.ndarray:
    global LAST_RESULT
    assert embeddings.shape == (B, N, D)
    assert labels.shape == (B, N)

    X = np.asarray(embeddings, dtype=np.float32).reshape(ROWS, D)
    lab = np.asarray(labels).reshape(ROWS)

    norms = np.sqrt(np.square(X).sum(axis=1, dtype=np.float32))
    Xq = (X * (SCALE / np.maximum(norms, np.float32(1e-12)))[:, None]).astype(NP_FP8)

    # NSTAR: max per-group negative count after balanced packing, across
    # all cores (the SPMD schedule is shared), rounded up to 8 (matmul gap
    # measurements: misaligned moving offsets slow streaming ~60%).
    lab_s = lab.reshape(-1, N)
    negs_all = (lab_s == 0).sum(axis=1)
    worst = 0
    packs = []
    for c in range(N_CORES):
        negs = negs_all[c * SAMPLES_PER_CORE:(c + 1) * SAMPLES_PER_CORE]
        gidx = _pack_groups(negs)
        packs.append(gidx)
        worst = max(worst, int(negs[gidx].sum(axis=1).max()))
    nstar = min(128, max(16, -(-worst // 8) * 8))

    in_maps, mgs = [], []
    for c in range(N_CORES):
        m, mg = _prep_core(Xq, lab, c, nstar, packs[c])
        in_maps.append(m)
        mgs.append(mg)

    if ("nc", nstar) not in _CACHE:
        _CACHE[("nc", nstar)] = _build_program(nstar)
    nc = _CACHE[("nc", nstar)]

    trace = os.environ.get("BASS_KERNEL_TRACE", "0") == "1"
    res = run_bass_kernel_spmd(nc, in_maps, list(range(N_CORES)), trace=trace)
    LAST_RESULT = res

    # out[p, g]: group g, stationary col (= permuted row) p
    inv_s2 = np.float64(1.0 / (SCALE * SCALE))
    loss_sum = 0.0
    for c in range(N_CORES):
        mx = np.asarray(res.results[c]["out"], dtype=np.float64).T  # [GROUPS,128]
        keep = np.arange(128)[None, :] < mgs[c][:, None]
        trip = np.maximum(mx * inv_s2 + np.float64(MARGIN), 0.0)
        loss_sum += float((trip * keep).sum())

    lab2 = np.asarray(labels)
    pos = lab2 == 1
    valid = pos.any(axis=1) & (lab2 == 0).any(axis=1)
    count = int((pos & valid[:, None]).sum())
    loss = np.float32(loss_sum / max(count, 1))
    return np.asarray(loss, dtype=np.float32)


# revision 15
# speedup vs baseline: 1.0076x; 1.0076x over previous
"""Trainium2 Bass kernel for CFContrastiveLoss.

Reference semantics (per sample of N=16 options, D=768 dims):
  - L2-normalize option embeddings
  - sim = pairwise cosine sims within the sample (16x16 gram)
  - max_neg[n] = max over negative-labeled columns of sim[n, :]
  - loss = mean over (positive rows of valid samples) of relu(max_neg + 0.3)

Device strategy (pure data parallel over batch, 8 cores):
  - The loss is a global mean, so samples can be assigned to any
    (core, group) slot.  The host packs 8 samples = 128 rows per group
    (128 groups per core), balancing per-group negative counts, and
    permutes each group's rows to [positives | negatives].
  - Labels are known before the program is built (built per call), so
    the schedule uses a fixed moving window: stationary = all 128 rows
    (full-width weights keep FWL on), moving = the last NSTAR columns
    (NSTAR = max negatives in any group, ~66).  This nearly halves
    TensorE streaming vs a full 128x128 gram: only pos-row x neg-col
    sims are computed.  Pos rows caught inside the window are masked.
  - Embeddings are host-normalized, scaled by 8 (a power of two so the
    final division is exact) and quantized to fp8 e4m3; measured
    end-to-end loss rel-err ~1.4e-4 (errors average out over ~52k
    contributing rows).  fp8 halves HBM traffic vs fp16 - the kernel
    is DMA-dispatch-bound.  Matmuls run in normal mode (DoubleRow
    disables FWL and LDWEIGHTS would dominate at our moving size).
  - DMA: only SP/Activation have hardware DGE queues (gpsimd's software
    queue serializes at ~300ns/packet - measured).  Queue dispatch costs
    ~18ns + ~3ns/KB per per-partition line, so each queue carries a
    64-partition range with full 24 KiB lines (super-groups of 32
    groups).  The output is accumulated in one SBUF tile and written by
    a single end-of-kernel DMA per queue half: a per-super-group
    [128 x 32] f32 write would cost 128 tiny packets each (~2.4us of
    queue time) - measured 18us of dispatch in an earlier revision.
  - Masking is folded into the PSUM accumulation as one extra K=9
    matmul of +-128 sentinel outer products (fp8-exact powers of two):
      row 0:   ones x (-128 * ones)            (mask everything ...)
      row 1+s: u_s  x (+128 * v_s)             (... except same-sample
                                                real-negative columns)
    u_s = rows of sample-slot s, v_s = real-negative moving columns of
    sample-slot s.  Sentinels cancel exactly in fp32 PSUM, so kept sims
    are bit-exact; masked entries sit <= sim-128 <= -64 and the host
    relu(max/64 + 0.3) kills them (invalid samples come out 0 for free).
  - Per group one VectorE row-max from PSUM; relu/weight/mean on host.
"""

import os

import numpy as np
import ml_dtypes

import concourse.bass as bass
import concourse.mybir as mybir
from concourse import bacc, tile
from concourse.bass_utils import run_bass_kernel_spmd

FP8 = mybir.dt.float8e4
F32 = mybir.dt.float32
NP_FP8 = ml_dtypes.float8_e4m3

B, N, D = 8192, 16, 768
N_CORES = 8
ROWS = B * N                      # 131072
ROWS_PER_CORE = ROWS // N_CORES   # 16384
SAMPLES_PER_CORE = ROWS_PER_CORE // N   # 1024
GROUPS = ROWS_PER_CORE // 128     # 128 groups of 128 rows (8 samples)
SPG = 128 // N                    # 8 samples per group
KCH = D // 128                    # 6 contraction chunks
SG = 8                            # groups per super-group (one DMA batch)
N_SG = GROUPS // SG               # 16
MASK_K = 1 + SPG                  # 9 live mask matmul rows
SCALE = np.float32(8.0)           # fp8 pre-scale (power of two)
SENT = np.float32(128.0)          # fp8-exact sentinel, > 1.3 * SCALE^2
MARGIN = np.float32(0.3)

_CACHE: dict = {}

LAST_RESULT = None  # BassKernelResults of the most recent device run


def _build_program(nstar: int) -> bass.Bass:
    nc = bacc.Bacc(None)
    et = nc.declare_dram_parameter("et", [N_SG, 128, SG * D], FP8, isOutput=False)
    mk = nc.declare_dram_parameter(
        "mk", [N_SG, MASK_K, SG * (128 + nstar)], FP8, isOutput=False)
    out = nc.declare_dram_parameter("out", [128, GROUPS], F32, isOutput=True)

    mv0 = 128 - nstar  # first moving column within each group

    with tile.TileContext(nc) as tc:
        with (
            tc.tile_pool(name="emb", bufs=6) as emb_pool,
            tc.tile_pool(name="const", bufs=1) as const_pool,
            tc.tile_pool(name="psum", bufs=8, space="PSUM") as psum_pool,
        ):
            # Ping-pong mask tiles (lhs cols | rhs cols per group, one DMA),
            # zero-padded to K=128: a K=9 mask matmul stalls the PE ~100ns
            # per group (stationary partition-size reconfig); dead
            # contraction rows are free.  Memset once at start.
            GW = 128 + nstar
            mk_tiles = []
            for i in range(2):
                mk_t = const_pool.tile([128, SG * GW], FP8, name=f"mk{i}")
                nc.vector.memset(mk_t[:, :], 0.0)
                mk_tiles.append(mk_t)
            wide = const_pool.tile([128, GROUPS], F32, name="wide")

            # Tile-framework dependencies are TILE-granular (a tile's first
            # matmul waits for every DMA touching the tile), so each tile is
            # one whole-tile descriptor on one queue, tiles alternating
            # between the two hardware queues; the tile's mask load rides
            # the opposite queue.  128 x 6144B lines per descriptor is the
            # measured per-queue sweet spot, and the 16 shared DMA engines
            # (~25 GB/s per packet stream) serve both queues concurrently.
            queues = [nc.sync, nc.scalar]
            for sg in range(N_SG):
                hi = emb_pool.tile([128, SG * D], FP8, tag="hi")
                mk_t = mk_tiles[sg % 2]
                queues[(sg + 1) % 2].dma_start(mk_t[:MASK_K, :], mk[sg])
                queues[sg % 2].dma_start(hi[:, :], et[sg][:, :])
                for gi in range(SG):
                    ps = psum_pool.tile([128, 512], F32)  # one full PSUM bank
                    G = ps[:, 0:nstar]
                    # Mask sentinels first (start=True clears the bank).
                    nc.tensor.matmul(
                        G,
                        mk_t[:, gi * GW:gi * GW + 128],
                        mk_t[:, gi * GW + 128:(gi + 1) * GW],
                        start=True, stop=False,
                    )
                    for k in range(KCH):
                        c0 = (gi * KCH + k) * 128
                        hk = hi[:, c0:c0 + 128]
                        nc.tensor.matmul(
                            G, hk, hk[:, mv0:128], start=False, stop=(k == KCH - 1))
                    g = sg * SG + gi
                    nc.vector.reduce_max(
                        wide[:, g:g + 1], G, axis=mybir.AxisListType.X)
            # Single output write, split across both hardware queues
            # (tiny per-partition lines make per-super-group writes cost
            # ~128 dispatch slots each).
            nc.sync.dma_start(out[0:64, :], wide[0:64, :])
            nc.scalar.dma_start(out[64:128, :], wide[64:128, :])
    nc.finalize()
    return nc


def _pack_groups(negs: np.ndarray) -> np.ndarray:
    """Assign SAMPLES_PER_CORE samples to GROUPS bins of SPG, balancing
    per-bin negative-row totals (greedy LPT).  Returns [GROUPS, SPG].
    Full bins leave the heap (only re-pushed while below capacity), and
    capacity exactly matches the sample count, so the pop always finds
    a non-full bin."""
    import heapq

    order = np.argsort(-negs, kind="stable")
    heap = [(0, g) for g in range(GROUPS)]
    heapq.heapify(heap)
    bins = [[] for _ in range(GROUPS)]
    for i in order:
        tot, g = heapq.heappop(heap)
        bins[g].append(i)
        if len(bins[g]) < SPG:
            heapq.heappush(heap, (tot + int(negs[i]), g))
    return np.array(bins, dtype=np.int64)


def _prep_core(Xq: np.ndarray, lab: np.ndarray, c: int, nstar: int,
               gidx: np.ndarray):
    """Per-core input map.  Xq: [ROWS, D] fp8 (normalized*SCALE), lab flat."""
    r0 = c * ROWS_PER_CORE
    lab_c = lab[r0:r0 + ROWS_PER_CORE].reshape(SAMPLES_PER_CORE, N)

    rows = (gidx[:, :, None] * N + np.arange(N)).reshape(GROUPS, 128)
    glab = lab_c.reshape(-1)[rows]                         # [GROUPS, 128]
    negflag = glab == 0
    order = np.argsort(negflag, axis=1, kind="stable")     # pos first
    prow = np.take_along_axis(rows, order, axis=1)         # [GROUPS, 128]
    mg = (~negflag).sum(axis=1)                            # pos count per group

    sampslot = np.broadcast_to(np.arange(128) // N, (GROUPS, 128))
    samp_p = np.take_along_axis(sampslot, order, axis=1)   # sample slot per col
    isneg_p = np.take_along_axis(negflag, order, axis=1)

    # mask lhsT rows: row 0 = ones; row 1+s = [col is sample-slot s]
    onehot = (samp_p[:, None, :] == np.arange(SPG)[None, :, None])
    mlhs = np.empty((GROUPS, MASK_K, 128), dtype=np.float32)
    mlhs[:, 0, :] = 1.0
    mlhs[:, 1:, :] = onehot
    # mask rhs rows (last nstar cols): row 0 = -SENT;
    # row 1+s = +SENT * [real-negative col of sample-slot s]
    mv0 = 128 - nstar
    mrhs = np.empty((GROUPS, MASK_K, nstar), dtype=np.float32)
    mrhs[:, 0, :] = -SENT
    mrhs[:, 1:, :] = SENT * (onehot[:, :, mv0:] & isneg_p[:, None, mv0:])

    Xp = Xq[r0 + prow]                                     # [GROUPS, 128, D] fp8
    et = np.ascontiguousarray(
        Xp.reshape(N_SG, SG, 128, KCH, 128).transpose(0, 4, 1, 3, 2)
    ).reshape(N_SG, 128, SG * D)
    mkc = np.concatenate([mlhs, mrhs], axis=2)             # [GROUPS, MASK_K, 128+nstar]
    mk8 = np.ascontiguousarray(
        mkc.astype(NP_FP8).reshape(N_SG, SG, MASK_K, 128 + nstar).transpose(0, 2, 1, 3)
    ).reshape(N_SG, MASK_K, SG * (128 + nstar))
    return {"et": et, "mk": mk8}, mg


def kernel(embeddings: np.ndarray, labels: np.ndarray) -> np.ndarray:
    global LAST_RESULT
    assert embeddings.shape == (B, N, D)
    assert labels.shape == (B, N)

    X = np.asarray(embeddings, dtype=np.float32).reshape(ROWS, D)
    lab = np.asarray(labels).reshape(ROWS)

    norms = np.sqrt(np.square(X).sum(axis=1, dtype=np.float32))
    Xq = (X * (SCALE / np.maximum(norms, np.float32(1e-12)))[:, None]).astype(NP_FP8)

    # NSTAR: max per-group negative count after balanced packing, across
    # all cores (the SPMD schedule is shared), rounded up to 8 (matmul gap
    # measurements: misaligned moving offsets slow streaming ~60%).
    lab_s = lab.reshape(-1, N)
    negs_all = (lab_s == 0).sum(axis=1)
    worst = 0
    packs = []
    for c in range(N_CORES):
        negs = negs_all[c * SAMPLES_PER_CORE:(c + 1) * SAMPLES_PER_CORE]
        gidx = _pack_groups(negs)
        packs.append(gidx)
        worst = max(worst, int(negs[gidx].sum(axis=1).max()))
    nstar = min(128, max(16, -(-worst // 8) * 8))

    in_maps, mgs = [], []
    for c in range(N_CORES):
        m, mg = _prep_core(Xq, lab, c, nstar, packs[c])
        in_maps.append(m)
        mgs.append(mg)

    if ("nc", nstar) not in _CACHE:
        _CACHE[("nc", nstar)] = _build_program(nstar)
    nc = _CACHE[("nc", nstar)]

    trace = os.environ.get("BASS_KERNEL_TRACE", "0") == "1"
    res = run_bass_kernel_spmd(nc, in_maps, list(range(N_CORES)), trace=trace)
    LAST_RESULT = res

    # out[p, g]: group g, stationary col (= permuted row) p
    inv_s2 = np.float64(1.0 / (SCALE * SCALE))
    loss_sum = 0.0
    for c in range(N_CORES):
        mx = np.asarray(res.results[c]["out"], dtype=np.float64).T  # [GROUPS,128]
        keep = np.arange(128)[None, :] < mgs[c][:, None]
        trip = np.maximum(mx * inv_s2 + np.float64(MARGIN), 0.0)
        loss_sum += float((trip * keep).sum())

    lab2 = np.asarray(labels)
    pos = lab2 == 1
    valid = pos.any(axis=1) & (lab2 == 0).any(axis=1)
    count = int((pos & valid[:, None]).sum())
    loss = np.float32(loss_sum / max(count, 1))
    return np.asarray(loss, dtype=np.float32)


# revision 16
# speedup vs baseline: 1.0390x; 1.0312x over previous
"""Trainium2 Bass kernel for CFContrastiveLoss.

Reference semantics (per sample of N=16 options, D=768 dims):
  - L2-normalize option embeddings
  - sim = pairwise cosine sims within the sample (16x16 gram)
  - max_neg[n] = max over negative-labeled columns of sim[n, :]
  - loss = mean over (positive rows of valid samples) of relu(max_neg + 0.3)

Device strategy (pure data parallel over batch, 8 cores):
  - The loss is a global mean, so samples can be assigned to any
    (core, group) slot.  The host packs 8 samples = 128 rows per group
    (128 groups per core), balancing per-group negative counts, and
    permutes each group's rows to [positives | negatives].
  - Labels are known before the program is built (built per call), so
    the schedule uses a fixed moving window: stationary = all 128 rows
    (full-width weights keep FWL on), moving = the last NSTAR columns
    (NSTAR = max negatives in any group, ~66).  This nearly halves
    TensorE streaming vs a full 128x128 gram: only pos-row x neg-col
    sims are computed.  Pos rows caught inside the window are masked.
  - Embeddings are host-normalized, scaled by 8 (a power of two so the
    final division is exact) and quantized to fp8 e4m3; measured
    end-to-end loss rel-err ~1.4e-4 (errors average out over ~52k
    contributing rows).  fp8 halves HBM traffic vs fp16 - the kernel
    is DMA-dispatch-bound.  Matmuls run in normal mode (DoubleRow
    disables FWL and LDWEIGHTS would dominate at our moving size).
  - DMA: only SP/Activation have hardware DGE queues (gpsimd's software
    queue serializes at ~300ns/packet - measured).  Queue dispatch costs
    ~18ns + ~3ns/KB per per-partition line, so each queue carries a
    64-partition range with full 24 KiB lines (super-groups of 32
    groups).  The output is accumulated in one SBUF tile and written by
    a single end-of-kernel DMA per queue half: a per-super-group
    [128 x 32] f32 write would cost 128 tiny packets each (~2.4us of
    queue time) - measured 18us of dispatch in an earlier revision.
  - Masking is folded into the PSUM accumulation as one extra K=9
    matmul of +-128 sentinel outer products (fp8-exact powers of two):
      row 0:   ones x (-128 * ones)            (mask everything ...)
      row 1+s: u_s  x (+128 * v_s)             (... except same-sample
                                                real-negative columns)
    u_s = rows of sample-slot s, v_s = real-negative moving columns of
    sample-slot s.  Sentinels cancel exactly in fp32 PSUM, so kept sims
    are bit-exact; masked entries sit <= sim-128 <= -64 and the host
    relu(max/64 + 0.3) kills them (invalid samples come out 0 for free).
  - Per group one VectorE row-max from PSUM; relu/weight/mean on host.
"""

import os

import numpy as np
import ml_dtypes

import concourse.bass as bass
import concourse.mybir as mybir
from concourse import bacc, tile
from concourse.bass_utils import run_bass_kernel_spmd

FP8 = mybir.dt.float8e4
F32 = mybir.dt.float32
NP_FP8 = ml_dtypes.float8_e4m3

B, N, D = 8192, 16, 768
N_CORES = 8
ROWS = B * N                      # 131072
ROWS_PER_CORE = ROWS // N_CORES   # 16384
SAMPLES_PER_CORE = ROWS_PER_CORE // N   # 1024
GROUPS = ROWS_PER_CORE // 128     # 128 groups of 128 rows (8 samples)
SPG = 128 // N                    # 8 samples per group
KCH = D // 128                    # 6 contraction chunks
SG = 8                            # groups per super-group (one DMA batch)
N_SG = GROUPS // SG               # 16
MASK_K = 1 + SPG                  # 9 live mask matmul rows
SCALE = np.float32(8.0)           # fp8 pre-scale (power of two)
SENT = np.float32(128.0)          # fp8-exact sentinel, > 1.3 * SCALE^2
MARGIN = np.float32(0.3)

_CACHE: dict = {}

LAST_RESULT = None  # BassKernelResults of the most recent device run


def _build_program(nstar: int) -> bass.Bass:
    nc = bacc.Bacc(None)
    et = nc.declare_dram_parameter("et", [N_SG, 128, SG * D], FP8, isOutput=False)
    mk = nc.declare_dram_parameter(
        "mk", [N_SG, MASK_K, SG * (128 + nstar)], FP8, isOutput=False)
    out = nc.declare_dram_parameter("out", [128, GROUPS], F32, isOutput=True)

    mv0 = 128 - nstar  # first moving column within each group

    with tile.TileContext(nc) as tc:
        with (
            tc.tile_pool(name="emb", bufs=6) as emb_pool,
            tc.tile_pool(name="const", bufs=1) as const_pool,
            tc.tile_pool(name="psum", bufs=8, space="PSUM") as psum_pool,
        ):
            # Ping-pong mask tiles (lhs cols | rhs cols per group, one DMA),
            # zero-padded to K=128: a K=9 mask matmul stalls the PE ~100ns
            # per group (stationary partition-size reconfig); dead
            # contraction rows are free.  Memset once at start.
            GW = 128 + nstar
            NMK = 4
            mk_tiles = []
            for i in range(NMK):
                mk_t = const_pool.tile([128, SG * GW], FP8, name=f"mk{i}")
                nc.vector.memset(mk_t[:, :], 0.0)
                mk_tiles.append(mk_t)
            wide = const_pool.tile([128, GROUPS], F32, name="wide")

            # Tile-framework dependencies are TILE-granular (a tile's first
            # matmul waits for every DMA touching the tile), so each tile is
            # one whole-tile descriptor on one queue, tiles alternating
            # between the two hardware queues; the tile's mask load rides
            # the opposite queue.  128 x 6144B lines per descriptor is the
            # measured per-queue sweet spot, and the 16 shared DMA engines
            # (~25 GB/s per packet stream) serve both queues concurrently.
            queues = [nc.sync, nc.scalar]
            # Prefetch masks NMK tiles deep: with only two ping-pong mask
            # buffers the PE stalled every other tile waiting for a mask
            # load posted behind a ~5us-deep queue (measured 8 stalls,
            # 12us total).
            for i in range(NMK):
                queues[(i + 1) % 2].dma_start(mk_tiles[i][:MASK_K, :], mk[i])
            for sg in range(N_SG):
                hi = emb_pool.tile([128, SG * D], FP8, tag="hi")
                mk_t = mk_tiles[sg % NMK]
                if sg + NMK < N_SG:
                    queues[(sg + NMK + 1) % 2].dma_start(
                        mk_tiles[(sg + NMK) % NMK][:MASK_K, :], mk[sg + NMK])
                queues[sg % 2].dma_start(hi[:, :], et[sg][:, :])
                for gi in range(SG):
                    ps = psum_pool.tile([128, 512], F32)  # one full PSUM bank
                    G = ps[:, 0:nstar]
                    # Mask sentinels first (start=True clears the bank).
                    nc.tensor.matmul(
                        G,
                        mk_t[:, gi * GW:gi * GW + 128],
                        mk_t[:, gi * GW + 128:(gi + 1) * GW],
                        start=True, stop=False,
                    )
                    for k in range(KCH):
                        c0 = (gi * KCH + k) * 128
                        hk = hi[:, c0:c0 + 128]
                        nc.tensor.matmul(
                            G, hk, hk[:, mv0:128], start=False, stop=(k == KCH - 1))
                    g = sg * SG + gi
                    nc.vector.reduce_max(
                        wide[:, g:g + 1], G, axis=mybir.AxisListType.X)
            # Single output write, split across both hardware queues
            # (tiny per-partition lines make per-super-group writes cost
            # ~128 dispatch slots each).
            nc.sync.dma_start(out[0:64, :], wide[0:64, :])
            nc.scalar.dma_start(out[64:128, :], wide[64:128, :])
    nc.finalize()
    return nc


def _pack_groups(negs: np.ndarray) -> np.ndarray:
    """Assign SAMPLES_PER_CORE samples to GROUPS bins of SPG, balancing
    per-bin negative-row totals (greedy LPT).  Returns [GROUPS, SPG].
    Full bins leave the heap (only re-pushed while below capacity), and
    capacity exactly matches the sample count, so the pop always finds
    a non-full bin."""
    import heapq

    order = np.argsort(-negs, kind="stable")
    heap = [(0, g) for g in range(GROUPS)]
    heapq.heapify(heap)
    bins = [[] for _ in range(GROUPS)]
    for i in order:
        tot, g = heapq.heappop(heap)
        bins[g].append(i)
        if len(bins[g]) < SPG:
            heapq.heappush(heap, (tot + int(negs[i]), g))
    return np.array(bins, dtype=np.int64)


def _prep_core(Xq: np.ndarray, lab: np.ndarray, c: int, nstar: int,
               gidx: np.ndarray):
    """Per-core input map.  Xq: [ROWS, D] fp8 (normalized*SCALE), lab flat."""
    r0 = c * ROWS_PER_CORE
    lab_c = lab[r0:r0 + ROWS_PER_CORE].reshape(SAMPLES_PER_CORE, N)

    rows = (gidx[:, :, None] * N + np.arange(N)).reshape(GROUPS, 128)
    glab = lab_c.reshape(-1)[rows]                         # [GROUPS, 128]
    negflag = glab == 0
    order = np.argsort(negflag, axis=1, kind="stable")     # pos first
    prow = np.take_along_axis(rows, order, axis=1)         # [GROUPS, 128]
    mg = (~negflag).sum(axis=1)                            # pos count per group

    sampslot = np.broadcast_to(np.arange(128) // N, (GROUPS, 128))
    samp_p = np.take_along_axis(sampslot, order, axis=1)   # sample slot per col
    isneg_p = np.take_along_axis(negflag, order, axis=1)

    # mask lhsT rows: row 0 = ones; row 1+s = [col is sample-slot s]
    onehot = (samp_p[:, None, :] == np.arange(SPG)[None, :, None])
    mlhs = np.empty((GROUPS, MASK_K, 128), dtype=np.float32)
    mlhs[:, 0, :] = 1.0
    mlhs[:, 1:, :] = onehot
    # mask rhs rows (last nstar cols): row 0 = -SENT;
    # row 1+s = +SENT * [real-negative col of sample-slot s]
    mv0 = 128 - nstar
    mrhs = np.empty((GROUPS, MASK_K, nstar), dtype=np.float32)
    mrhs[:, 0, :] = -SENT
    mrhs[:, 1:, :] = SENT * (onehot[:, :, mv0:] & isneg_p[:, None, mv0:])

    Xp = Xq[r0 + prow]                                     # [GROUPS, 128, D] fp8
    et = np.ascontiguousarray(
        Xp.reshape(N_SG, SG, 128, KCH, 128).transpose(0, 4, 1, 3, 2)
    ).reshape(N_SG, 128, SG * D)
    mkc = np.concatenate([mlhs, mrhs], axis=2)             # [GROUPS, MASK_K, 128+nstar]
    mk8 = np.ascontiguousarray(
        mkc.astype(NP_FP8).reshape(N_SG, SG, MASK_K, 128 + nstar).transpose(0, 2, 1, 3)
    ).reshape(N_SG, MASK_K, SG * (128 + nstar))
    return {"et": et, "mk": mk8}, mg


def kernel(embeddings: np.ndarray, labels: np.ndarray) -> np.ndarray:
    global LAST_RESULT
    assert embeddings.shape == (B, N, D)
    assert labels.shape == (B, N)

    X = np.asarray(embeddings, dtype=np.float32).reshape(ROWS, D)
    lab = np.asarray(labels).reshape(ROWS)

    norms = np.sqrt(np.square(X).sum(axis=1, dtype=np.float32))
    Xq = (X * (SCALE / np.maximum(norms, np.float32(1e-12)))[:, None]).astype(NP_FP8)

    # NSTAR: max per-group negative count after balanced packing, across
    # all cores (the SPMD schedule is shared), rounded up to 8 (matmul gap
    # measurements: misaligned moving offsets slow streaming ~60%).
    lab_s = lab.reshape(-1, N)
    negs_all = (lab_s == 0).sum(axis=1)
    worst = 0
    packs = []
    for c in range(N_CORES):
        negs = negs_all[c * SAMPLES_PER_CORE:(c + 1) * SAMPLES_PER_CORE]
        gidx = _pack_groups(negs)
        packs.append(gidx)
        worst = max(worst, int(negs[gidx].sum(axis=1).max()))
    nstar = min(128, max(16, -(-worst // 8) * 8))

    in_maps, mgs = [], []
    for c in range(N_CORES):
        m, mg = _prep_core(Xq, lab, c, nstar, packs[c])
        in_maps.append(m)
        mgs.append(mg)

    if ("nc", nstar) not in _CACHE:
        _CACHE[("nc", nstar)] = _build_program(nstar)
    nc = _CACHE[("nc", nstar)]

    trace = os.environ.get("BASS_KERNEL_TRACE", "0") == "1"
    res = run_bass_kernel_spmd(nc, in_maps, list(range(N_CORES)), trace=trace)
    LAST_RESULT = res

    # out[p, g]: group g, stationary col (= permuted row) p
    inv_s2 = np.float64(1.0 / (SCALE * SCALE))
    loss_sum = 0.0
    for c in range(N_CORES):
        mx = np.asarray(res.results[c]["out"], dtype=np.float64).T  # [GROUPS,128]
        keep = np.arange(128)[None, :] < mgs[c][:, None]
        trip = np.maximum(mx * inv_s2 + np.float64(MARGIN), 0.0)
        loss_sum += float((trip * keep).sum())

    lab2 = np.asarray(labels)
    pos = lab2 == 1
    valid = pos.any(axis=1) & (lab2 == 0).any(axis=1)
    count = int((pos & valid[:, None]).sum())
    loss = np.float32(loss_sum / max(count, 1))
    return np.asarray(loss, dtype=np.float32)


# revision 17
# speedup vs baseline: 1.2054x; 1.1601x over previous
"""Trainium2 Bass kernel for CFContrastiveLoss.

Reference semantics (per sample of N=16 options, D=768 dims):
  - L2-normalize option embeddings
  - sim = pairwise cosine sims within the sample (16x16 gram)
  - max_neg[n] = max over negative-labeled columns of sim[n, :]
  - loss = mean over (positive rows of valid samples) of relu(max_neg + 0.3)

Device strategy (pure data parallel over batch, 8 cores):
  - The loss is a global mean, so samples can be assigned to any
    (core, group) slot.  The host packs 8 samples = 128 rows per group
    (128 groups per core), balancing per-group negative counts, and
    permutes each group's rows to [positives | negatives].
  - Labels are known before the program is built (built per call), so
    the schedule uses a fixed moving window: stationary = all 128 rows
    (full-width weights keep FWL on), moving = the last NSTAR columns
    (NSTAR = max negatives in any group, ~66).  This nearly halves
    TensorE streaming vs a full 128x128 gram: only pos-row x neg-col
    sims are computed.  Pos rows caught inside the window are masked.
  - Embeddings are host-normalized, scaled by 8 (a power of two so the
    final division is exact) and quantized to fp8 e4m3; measured
    end-to-end loss rel-err ~1.4e-4 (errors average out over ~52k
    contributing rows).  fp8 halves HBM traffic vs fp16 - the kernel
    is DMA-dispatch-bound.  Matmuls run in normal mode (DoubleRow
    disables FWL and LDWEIGHTS would dominate at our moving size).
  - DMA: only SP/Activation have hardware DGE queues (gpsimd's software
    queue serializes at ~300ns/packet - measured).  Queue dispatch costs
    ~18ns + ~3ns/KB per per-partition line, so each queue carries a
    64-partition range with full 24 KiB lines (super-groups of 32
    groups).  The output is accumulated in one SBUF tile and written by
    a single end-of-kernel DMA per queue half: a per-super-group
    [128 x 32] f32 write would cost 128 tiny packets each (~2.4us of
    queue time) - measured 18us of dispatch in an earlier revision.
  - Masking is folded into the PSUM accumulation as one extra K=9
    matmul of +-128 sentinel outer products (fp8-exact powers of two):
      row 0:   ones x (-128 * ones)            (mask everything ...)
      row 1+s: u_s  x (+128 * v_s)             (... except same-sample
                                                real-negative columns)
    u_s = rows of sample-slot s, v_s = real-negative moving columns of
    sample-slot s.  Sentinels cancel exactly in fp32 PSUM, so kept sims
    are bit-exact; masked entries sit <= sim-128 <= -64 and the host
    relu(max/64 + 0.3) kills them (invalid samples come out 0 for free).
  - Per group one VectorE row-max from PSUM; relu/weight/mean on host.
"""

import os

import numpy as np
import ml_dtypes

import concourse.bass as bass
import concourse.mybir as mybir
from concourse import bacc, tile
from concourse.bass_utils import run_bass_kernel_spmd

FP8 = mybir.dt.float8e4
F32 = mybir.dt.float32
NP_FP8 = ml_dtypes.float8_e4m3

B, N, D = 8192, 16, 768
N_CORES = 8
ROWS = B * N                      # 131072
ROWS_PER_CORE = ROWS // N_CORES   # 16384
SAMPLES_PER_CORE = ROWS_PER_CORE // N   # 1024
GROUPS = ROWS_PER_CORE // 128     # 128 groups of 128 rows (8 samples)
SPG = 128 // N                    # 8 samples per group
KCH = D // 128                    # 6 contraction chunks
SG = 8                            # groups per super-group (one DMA batch)
N_SG = GROUPS // SG               # 16
MASK_K = 1 + SPG                  # 9 live mask matmul rows
SCALE = np.float32(8.0)           # fp8 pre-scale (power of two)
SENT = np.float32(128.0)          # fp8-exact sentinel, > 1.3 * SCALE^2
MARGIN = np.float32(0.3)

_CACHE: dict = {}

LAST_RESULT = None  # BassKernelResults of the most recent device run


def _build_program(nstar: int) -> bass.Bass:
    nc = bacc.Bacc(None)
    et = nc.declare_dram_parameter("et", [N_SG, 128, SG * D], FP8, isOutput=False)
    mk = nc.declare_dram_parameter(
        "mk", [N_SG, MASK_K, SG * (128 + nstar)], FP8, isOutput=False)
    out = nc.declare_dram_parameter("out", [128, GROUPS], F32, isOutput=True)

    mv0 = 128 - nstar  # first moving column within each group

    with tile.TileContext(nc) as tc:
        with (
            tc.tile_pool(name="emb", bufs=6) as emb_pool,
            tc.tile_pool(name="const", bufs=1) as const_pool,
            tc.tile_pool(name="psum", bufs=8, space="PSUM") as psum_pool,
        ):
            # Ping-pong mask tiles (lhs cols | rhs cols per group, one DMA),
            # zero-padded to K=128: a K=9 mask matmul stalls the PE ~100ns
            # per group (stationary partition-size reconfig); dead
            # contraction rows are free.  Memset once at start.
            GW = 128 + nstar
            NMK = 4
            mk_tiles = []
            for i in range(NMK):
                mk_t = const_pool.tile([128, SG * GW], FP8, name=f"mk{i}")
                nc.vector.memset(mk_t[:, :], 0.0)
                mk_tiles.append(mk_t)
            wide = const_pool.tile([128, GROUPS], F32, name="wide")

            # Tile-framework dependencies are TILE-granular (a tile's first
            # matmul waits for every DMA touching the tile), so each tile is
            # one whole-tile descriptor on one queue, tiles alternating
            # between the two hardware queues; the tile's mask load rides
            # the opposite queue.  128 x 6144B lines per descriptor is the
            # measured per-queue sweet spot, and the 16 shared DMA engines
            # (~25 GB/s per packet stream) serve both queues concurrently.
            queues = [nc.sync, nc.scalar]
            # Prefetch masks NMK tiles deep: with only two ping-pong mask
            # buffers the PE stalled every other tile waiting for a mask
            # load posted behind a ~5us-deep queue (measured 8 stalls,
            # 12us total).
            for i in range(NMK):
                queues[(i + 1) % 2].dma_start(mk_tiles[i][:MASK_K, :], mk[i])
            for sg in range(N_SG):
                hi = emb_pool.tile([128, SG * D], FP8, tag="hi")
                mk_t = mk_tiles[sg % NMK]
                queues[sg % 2].dma_start(hi[:, :], et[sg][:, :])
                for gi in range(SG):
                    ps = psum_pool.tile([128, 512], F32)  # one full PSUM bank
                    G = ps[:, 0:nstar]
                    # Mask sentinels first (start=True clears the bank).
                    nc.tensor.matmul(
                        G,
                        mk_t[:, gi * GW:gi * GW + 128],
                        mk_t[:, gi * GW + 128:(gi + 1) * GW],
                        start=True, stop=False,
                    )
                    for k in range(KCH):
                        c0 = (gi * KCH + k) * 128
                        hk = hi[:, c0:c0 + 128]
                        nc.tensor.matmul(
                            G, hk, hk[:, mv0:128], start=False, stop=(k == KCH - 1))
                    g = sg * SG + gi
                    nc.vector.reduce_max(
                        wide[:, g:g + 1], G, axis=mybir.AxisListType.X)
                # Prefetch the mask NMK tiles ahead.  Posted AFTER this
                # tile's matmuls so the buffer overwrite orders after the
                # last read (posting it earlier raced and corrupted masks).
                if sg + NMK < N_SG:
                    queues[(sg + NMK + 1) % 2].dma_start(
                        mk_tiles[(sg + NMK) % NMK][:MASK_K, :], mk[sg + NMK])
            # Single output write, split across both hardware queues
            # (tiny per-partition lines make per-super-group writes cost
            # ~128 dispatch slots each).
            nc.sync.dma_start(out[0:64, :], wide[0:64, :])
            nc.scalar.dma_start(out[64:128, :], wide[64:128, :])
    nc.finalize()
    return nc


def _pack_groups(negs: np.ndarray) -> np.ndarray:
    """Assign SAMPLES_PER_CORE samples to GROUPS bins of SPG, balancing
    per-bin negative-row totals (greedy LPT).  Returns [GROUPS, SPG].
    Full bins leave the heap (only re-pushed while below capacity), and
    capacity exactly matches the sample count, so the pop always finds
    a non-full bin."""
    import heapq

    order = np.argsort(-negs, kind="stable")
    heap = [(0, g) for g in range(GROUPS)]
    heapq.heapify(heap)
    bins = [[] for _ in range(GROUPS)]
    for i in order:
        tot, g = heapq.heappop(heap)
        bins[g].append(i)
        if len(bins[g]) < SPG:
            heapq.heappush(heap, (tot + int(negs[i]), g))
    return np.array(bins, dtype=np.int64)


def _prep_core(Xq: np.ndarray, lab: np.ndarray, c: int, nstar: int,
               gidx: np.ndarray):
    """Per-core input map.  Xq: [ROWS, D] fp8 (normalized*SCALE), lab flat."""
    r0 = c * ROWS_PER_CORE
    lab_c = lab[r0:r0 + ROWS_PER_CORE].reshape(SAMPLES_PER_CORE, N)

    rows = (gidx[:, :, None] * N + np.arange(N)).reshape(GROUPS, 128)
    glab = lab_c.reshape(-1)[rows]                         # [GROUPS, 128]
    negflag = glab == 0
    order = np.argsort(negflag, axis=1, kind="stable")     # pos first
    prow = np.take_along_axis(rows, order, axis=1)         # [GROUPS, 128]
    mg = (~negflag).sum(axis=1)                            # pos count per group

    sampslot = np.broadcast_to(np.arange(128) // N, (GROUPS, 128))
    samp_p = np.take_along_axis(sampslot, order, axis=1)   # sample slot per col
    isneg_p = np.take_along_axis(negflag, order, axis=1)

    # mask lhsT rows: row 0 = ones; row 1+s = [col is sample-slot s]
    onehot = (samp_p[:, None, :] == np.arange(SPG)[None, :, None])
    mlhs = np.empty((GROUPS, MASK_K, 128), dtype=np.float32)
    mlhs[:, 0, :] = 1.0
    mlhs[:, 1:, :] = onehot
    # mask rhs rows (last nstar cols): row 0 = -SENT;
    # row 1+s = +SENT * [real-negative col of sample-slot s]
    mv0 = 128 - nstar
    mrhs = np.empty((GROUPS, MASK_K, nstar), dtype=np.float32)
    mrhs[:, 0, :] = -SENT
    mrhs[:, 1:, :] = SENT * (onehot[:, :, mv0:] & isneg_p[:, None, mv0:])

    Xp = Xq[r0 + prow]                                     # [GROUPS, 128, D] fp8
    et = np.ascontiguousarray(
        Xp.reshape(N_SG, SG, 128, KCH, 128).transpose(0, 4, 1, 3, 2)
    ).reshape(N_SG, 128, SG * D)
    mkc = np.concatenate([mlhs, mrhs], axis=2)             # [GROUPS, MASK_K, 128+nstar]
    mk8 = np.ascontiguousarray(
        mkc.astype(NP_FP8).reshape(N_SG, SG, MASK_K, 128 + nstar).transpose(0, 2, 1, 3)
    ).reshape(N_SG, MASK_K, SG * (128 + nstar))
    return {"et": et, "mk": mk8}, mg


def kernel(embeddings: np.ndarray, labels: np.ndarray) -> np.ndarray:
    global LAST_RESULT
    assert embeddings.shape == (B, N, D)
    assert labels.shape == (B, N)

    X = np.asarray(embeddings, dtype=np.float32).reshape(ROWS, D)
    lab = np.asarray(labels).reshape(ROWS)

    norms = np.sqrt(np.square(X).sum(axis=1, dtype=np.float32))
    Xq = (X * (SCALE / np.maximum(norms, np.float32(1e-12)))[:, None]).astype(NP_FP8)

    # NSTAR: max per-group negative count after balanced packing, across
    # all cores (the SPMD schedule is shared), rounded up to 8 (matmul gap
    # measurements: misaligned moving offsets slow streaming ~60%).
    lab_s = lab.reshape(-1, N)
    negs_all = (lab_s == 0).sum(axis=1)
    worst = 0
    packs = []
    for c in range(N_CORES):
        negs = negs_all[c * SAMPLES_PER_CORE:(c + 1) * SAMPLES_PER_CORE]
        gidx = _pack_groups(negs)
        packs.append(gidx)
        worst = max(worst, int(negs[gidx].sum(axis=1).max()))
    nstar = min(128, max(16, -(-worst // 8) * 8))

    in_maps, mgs = [], []
    for c in range(N_CORES):
        m, mg = _prep_core(Xq, lab, c, nstar, packs[c])
        in_maps.append(m)
        mgs.append(mg)

    if ("nc", nstar) not in _CACHE:
        _CACHE[("nc", nstar)] = _build_program(nstar)
    nc = _CACHE[("nc", nstar)]

    trace = os.environ.get("BASS_KERNEL_TRACE", "0") == "1"
    res = run_bass_kernel_spmd(nc, in_maps, list(range(N_CORES)), trace=trace)
    LAST_RESULT = res

    # out[p, g]: group g, stationary col (= permuted row) p
    inv_s2 = np.float64(1.0 / (SCALE * SCALE))
    loss_sum = 0.0
    for c in range(N_CORES):
        mx = np.asarray(res.results[c]["out"], dtype=np.float64).T  # [GROUPS,128]
        keep = np.arange(128)[None, :] < mgs[c][:, None]
        trip = np.maximum(mx * inv_s2 + np.float64(MARGIN), 0.0)
        loss_sum += float((trip * keep).sum())

    lab2 = np.asarray(labels)
    pos = lab2 == 1
    valid = pos.any(axis=1) & (lab2 == 0).any(axis=1)
    count = int((pos & valid[:, None]).sum())
    loss = np.float32(loss_sum / max(count, 1))
    return np.asarray(loss, dtype=np.float32)
